# revision 33
# baseline (speedup 1.0000x reference)
# Multi-head causal attention (B=2, T=2048, D=1024, H=16, HS=64) on 8 TRN2 NeuronCores.
#
# Sharding: core c = (batch b = c//4, head-group g = c%4 -> heads 4g..4g+3).
# Host slices w_qkv columns / w_out rows per core; each core computes a
# partial (T, D) output projection; host sums the 4 partials per batch and
# adds b_out plus the (exactly folded) V-bias term b_v @ w_out.
#
# v2 dataflow (per core) - everything quantized to fp8 e4m3, sized by the
# cost model's rule "matmul cost = out free columns x 0.5 (fp8 DoubleRow) /
# 1.0 (fp16)"; pointwise PSUM drains are the bottleneck and are split
# between ACT and DVE by a greedy load balancer:
#   - QKV projection: x8=Q(32x), w8=Q(32w) (QK) / Q(w) (V); 4 DoubleRow
#     matmuls per tile (no error-correction terms - quantization noise is
#     ~25x below the 2e-2 tolerance).
#   - Q^T,K^T land as fp8 in a DoubleRow-ready layout [128p=(head,hs%32),
#     2 hs-halves, 2 qk, T] via a host-side column permutation of w_qkv, so
#     score matmuls run fp8 DR with 32-partition k-tiles (PE quadrant
#     tile_position) at 0.5 cyc/col.
#   - Causal masking of diagonal blocks is done on the PE: a rank-structured
#     (-240*16) A^T B accumulation writes -3840 into masked score psum
#     entries (A=16*[j<=k], B=-240*[j==q+1]).
#   - exp: split ACT (native Exp -> fp8) / DVE (Schraudolph: I8 =
#     rne(s*8*log2e/128 + 55.63) written through an int8 bitcast of the fp8
#     pt tile; masked -3840 scores saturate to -128 = -0.0 in e4m3).
#   - PV: fp8 DoubleRow pairing two adjacent key blocks per instruction
#     (pt slot s holds key block s^1, matching vones slot s = V[s^1]).
#   - o normalized via one broadcast tensor_tensor per (head,qs), DMA-XBAR
#     transposed in 4-tile batches, projected in fp16, drained to fp16 SBUF
#     and DMA'd out.
import math
import os
import sys

import numpy as np

for _p in ("/opt/trn_rl_repo",):
    if _p not in sys.path and os.path.isdir(_p):
        sys.path.insert(0, _p)

import concourse.bass as bass
import concourse.mybir as mybir
import concourse.tile as tile
from concourse import bacc
from concourse import bass_utils

B, T, D = 2, 2048, 1024
H, HS = 16, 64
NCORES = 8
GROUPS = NCORES // B          # head-groups per batch = 4
HPC = H // GROUPS             # heads per core = 4
EC = HPC * HS                 # head-dim cols per section per core = 256
DC = D // 128                 # d-chunks = 8
TT = T // 128                 # t-tiles = 16
QS = 512                      # q-supertile
NQS = T // QS                 # 4
SCALE = 1.0 / math.sqrt(HS)

F32 = mybir.dt.float32
F16 = mybir.dt.float16
FP8 = mybir.dt.float8e4
I8 = mybir.dt.int8
U8 = mybir.dt.uint8
DR = mybir.MatmulPerfMode.DoubleRow
XS = 32.0                     # x fp8 pre-scale
WSQK = 32.0                   # w fp8 pre-scale for Q/K columns
QS2 = 4.0                     # stored qk8 = QS2 * q
DESC = QS2 / (XS * WSQK)      # psum -> qk8 scale = 1/256
SEXP = SCALE / (QS2 * QS2)    # exp scale on score psum = 1/128
VSC = 32.0                    # stored v = 32*v (w_v scale 1.0)
# fp16 Schraudolph exp: bits16 = rne(1024*log2(p)+1024*(15-C)); p = exp(psum/128)
# fits fp16 range directly (p max ~2840 << 65504); uint16 convert saturates
# the tiny-p negative-bits case to +0.0
A16 = 1024.0 * math.log2(math.e) * SEXP
B16 = 1024.0 * (15.0 - 0.043)

PTLAG = int(os.environ.get("PTLAG", "4"))  # flush deadline in units (< pt bufs - 1)
MULT = mybir.AluOpType.mult
ADD = mybir.AluOpType.add
EXP = mybir.ActivationFunctionType.Exp
COPY = mybir.ActivationFunctionType.Copy
IDENT = mybir.ActivationFunctionType.Identity


def _mha_tile_kernel(tc, outp, x8, xl, xh, w8, wql, wqh, wv8, wvl, wvh, wo, bqk, mska, with_bias):
    nc = tc.nc
    KS = int(os.environ.get("KS", "4"))

    # greedy ACT/DVE load balancer for PSUM-drain pointwise ops; Pool
    # (GPSIMD) takes SBUF-only work (causal mask multiplies)
    load = {"act": 0.0, "dve": 0.0, "pool": 0.0}

    def flex(act_emit, dve_emit, cols, act_ops=1, dve_ops=1, force=None):
        ca = cols * 0.833 + 185.0 * act_ops
        cd = cols * 1.042 + 125.0 * dve_ops
        eng = force or ("act" if load["act"] + ca <= load["dve"] + cd else "dve")
        if eng == "act":
            load["act"] += ca
            act_emit()
        else:
            load["dve"] += cd
            dve_emit()
        return ca if eng == "act" else cd

    with (
        tc.tile_pool(name="singles", bufs=1) as singles,
        tc.tile_pool(name="pt", bufs=5) as ptp,
        tc.tile_pool(name="rl", bufs=4) as rlp,
        tc.tile_pool(name="ob", bufs=4) as obp,
        tc.tile_pool(name="psum", bufs=1, space="PSUM") as psa,
    ):
        x8_sb = singles.tile([128, NQS, DC, QS], FP8)
        xl_sb = singles.tile([128, NQS, DC, QS], FP8)
        xh_sb = singles.tile([128, NQS, DC, QS], FP8)
        w8_sb = singles.tile([128, DC, 4, 128], FP8)
        wql_sb = singles.tile([128, DC, 4, 128], FP8)
        wqh_sb = singles.tile([128, DC, 4, 128], FP8)
        wv_sb = singles.tile([128, DC, EC], FP8)
        wvl_sb = singles.tile([128, DC, EC], FP8)
        wvh_sb = singles.tile([128, DC, EC], FP8)
        wo_sb = singles.tile([128, EC // 128, D], F16)
        msk_sb = singles.tile([128, 128], F16)
        bqk_sb = singles.tile([128, 4], F32)

        # loads: QK-critical first; x slabs on the gpsimd SWDGE queue so
        # descriptor generation overlaps the sync-queue loads.
        nc.sync.dma_start(out=w8_sb, in_=w8.rearrange("p (c e j) -> p c e j", c=DC, e=4))
        nc.sync.dma_start(out=wql_sb, in_=wql.rearrange("p (c e j) -> p c e j", c=DC, e=4))
        nc.sync.dma_start(out=wqh_sb, in_=wqh.rearrange("p (c e j) -> p c e j", c=DC, e=4))
        nc.sync.dma_start(out=x8_sb[:, 0], in_=x8.rearrange("p (s c t) -> p s c t", s=NQS, c=DC)[:, 0])
        nc.sync.dma_start(out=msk_sb, in_=mska)
        if with_bias:
            nc.sync.dma_start(out=bqk_sb, in_=bqk)
        nc.sync.dma_start(out=wv_sb, in_=wv8.rearrange("p (c e) -> p c e", c=DC))
        nc.sync.dma_start(out=wvl_sb, in_=wvl.rearrange("p (c e) -> p c e", c=DC))
        nc.sync.dma_start(out=wvh_sb, in_=wvh.rearrange("p (c e) -> p c e", c=DC))
        for s in range(1, NQS):
            nc.sync.dma_start(out=x8_sb[:, s], in_=x8.rearrange("p (s c t) -> p s c t", s=NQS, c=DC)[:, s])
        for s in range(NQS):
            nc.gpsimd.dma_start(out=xl_sb[:, s], in_=xl.rearrange("p (s c t) -> p s c t", s=NQS, c=DC)[:, s])
            nc.gpsimd.dma_start(out=xh_sb[:, s], in_=xh.rearrange("p (s c t) -> p s c t", s=NQS, c=DC)[:, s])
        nc.sync.dma_start(out=wo_sb, in_=wo.rearrange("p (c e) -> p c e", c=EC // 128))

        qk8_sb = singles.tile([128, 2, 2, T], F16)      # [p=(h%2)*64+hs, hpair, q/k, t]
        vones_sb = singles.tile([128, TT, HPC, HS + 1], F16)
        o_sb = singles.tile([128, EC // 128, TT, 128], F16)
        oT_sb = singles.tile([128, EC // 128, TT, 128], F16)
        nc.vector.memset(vones_sb[:, :, :, HS:HS + 1], VSC)

        def emit_qk_pair(qk, ts):
            # both hs-halves (2 ets) of q or k for one t-supertile
            ps = psa.tile([128, 1024], F32, tag="s", bufs=3, name="psqk")
            plan = ([(x8_sb, w8_sb)] * (DC // 2) + [(xl_sb, wqh_sb)] * (DC // 2)
                    + [(xh_sb, wql_sb)] * (DC // 2))
            for half in range(2):
                et = 2 * qk + half
                for i, (xt, wt) in enumerate(plan):
                    dp = i % (DC // 2)
                    nc.tensor.matmul(
                        ps[:, half * 512:(half + 1) * 512],
                        lhsT=wt[:, 2 * dp:2 * dp + 2, et, :],
                        rhs=xt[:, ts, 2 * dp:2 * dp + 2, :],
                        start=(i == 0), stop=(i == len(plan) - 1),
                        perf_mode=DR,
                    )
            psv = ps.rearrange("p (h t) -> p h t", h=2)
            outa = qk8_sb[:, :, qk, ts * QS:(ts + 1) * QS]
            # half = head-pair index; partitions carry (h%2)*64+hs directly
            if with_bias:
                for half in range(2):
                    et = 2 * qk + half
                    flex(
                        lambda h2=half, e2=et: nc.scalar.activation(
                            out=outa[:, h2], in_=psv[:, h2], func=IDENT,
                            scale=DESC, bias=bqk_sb[:, e2:e2 + 1]),
                        lambda h2=half, e2=et: nc.vector.tensor_scalar(
                            out=outa[:, h2], in0=psv[:, h2], scalar1=DESC,
                            scalar2=bqk_sb[:, e2:e2 + 1], op0=MULT, op1=ADD),
                        512,
                    )
            else:
                # two contiguous writes (one per hs-half slot): non-contiguous
                # strided writes have shown RAW races to downstream readers
                for half in range(2):
                    flex(
                        lambda h2=half: nc.scalar.activation(
                            out=outa[:, h2], in_=psv[:, h2], func=COPY, scale=DESC),
                        lambda h2=half: nc.vector.tensor_scalar(
                            out=outa[:, h2], in0=psv[:, h2], scalar1=DESC,
                            scalar2=0.0, op0=MULT, op1=ADD),
                        512,
                    )

        def emit_v_quad(tq):
            # V for t-tiles 4tq..4tq+3 (one x slab), written slot-swapped.
            # Exact 3-term fp8 split (drop only lo*lo): hi*hi + xl*wh + xh*wl
            ps = psa.tile([128, 4, EC], F32, tag="s", bufs=3, name="psv")
            plan = ([(x8_sb, wv_sb)] * (DC // 2) + [(xl_sb, wvh_sb)] * (DC // 2)
                    + [(xh_sb, wvl_sb)] * (DC // 2))
            for tw in range(4):
                for i, (xt, wt) in enumerate(plan):
                    dp = i % (DC // 2)
                    nc.tensor.matmul(
                        ps[:, tw ^ 1, :],
                        lhsT=xt[:, tq, 2 * dp:2 * dp + 2, tw * 128:(tw + 1) * 128],
                        rhs=wt[:, 2 * dp:2 * dp + 2, :],
                        start=(i == 0), stop=(i == len(plan) - 1),
                        perf_mode=DR,
                    )
            psv = ps.rearrange("p w (h s) -> p w h s", h=HPC)
            out = vones_sb[:, 4 * tq:4 * tq + 4, :, 0:HS]
            flex(
                lambda: nc.scalar.activation(out=out, in_=psv, func=COPY),
                lambda: nc.vector.tensor_copy(out=out, in_=psv),
                1024,
            )

        def flex_exp(pt_out, sps_in, cols, force=None):
            flex(
                lambda: nc.scalar.activation(out=pt_out, in_=sps_in, func=EXP,
                                             scale=SEXP),
                lambda: nc.vector.tensor_scalar(
                    out=pt_out.bitcast(mybir.dt.uint16), in0=sps_in,
                    scalar1=A16, scalar2=B16, op0=MULT, op1=ADD),
                cols,
                force=force or os.environ.get("FEXP") or None,
            )

        def emit_mask(pt, slot, c0):
            # causal boundary mask: zero pt[k > q] with a {0,1} fp8 multiply
            ap = pt[:, slot, c0:c0 + 128]
            load["dve"] += 128 * 0.261 + 125.0
            nc.vector.tensor_mul(out=ap, in0=ap, in1=msk_sb)

        def emit_scores(h, qs, pt, pace):
            # 64-row plain fp8 matmuls at partition base {0,64} (2 heads per
            # 128-partition group) - mixing sub-64-row PE tiles with 128-row
            # matmuls misroutes outputs on HW, so no DoubleRow here
            pb = 64 * (h % 2)
            kT = qk8_sb[pb:pb + 64, h // 2, 1, :]
            qT = qk8_sb[pb:pb + 64, h // 2, 0, qs * QS:(qs + 1) * QS]
            d0 = 4 * qs
            ptf = pt[:].rearrange("p a b -> p (a b)")

            for j2 in range(2 * qs):
                sps = psa.tile([128, 1024], F32, tag="s", bufs=3, name="sps")
                for half in range(2):
                    kb = 2 * j2 + (1 - half)  # slot s holds kb s^1
                    nc.tensor.matmul(
                        sps[:, half * 512:(half + 1) * 512],
                        lhsT=kT[:, kb * 128:(kb + 1) * 128], rhs=qT,
                        start=True, stop=True,
                    )
                flex_exp(pt[:, 2 * j2:2 * j2 + 2, :], sps, 1024)
                pace(1024.0)
            # diagonal pair A: slot d0 <- kb d0+1 (q 128:512), slot d0+1 <- kb d0 (q 0:512)
            sps = psa.tile([128, 1024], F32, tag="s", bufs=3, name="sps")
            nc.tensor.matmul(sps[:, 128:512],
                             lhsT=kT[:, (d0 + 1) * 128:(d0 + 2) * 128],
                             rhs=qT[:, 128:512], start=True, stop=True)
            nc.tensor.matmul(sps[:, 512:1024],
                             lhsT=kT[:, d0 * 128:(d0 + 1) * 128],
                             rhs=qT, start=True, stop=True)
            flex_exp(ptf[:, d0 * 512 + 128:(d0 + 2) * 512], sps[:, 128:1024], 896,
                     force="act" if qs == 0 else None)
            emit_mask(pt, d0, 128)       # kb d0+1 boundary (j=1)
            emit_mask(pt, d0 + 1, 0)     # kb d0 boundary (j=0)
            pace(896.0)
            # diagonal pair B: slot d0+2 <- kb d0+3 (q 384:512), slot d0+3 <- kb d0+2 (q 256:512)
            sps = psa.tile([128, 1024], F32, tag="s", bufs=3, name="sps")
            nc.tensor.matmul(sps[:, 384:512],
                             lhsT=kT[:, (d0 + 3) * 128:(d0 + 4) * 128],
                             rhs=qT[:, 384:512], start=True, stop=True)
            nc.tensor.matmul(sps[:, 512 + 256:1024],
                             lhsT=kT[:, (d0 + 2) * 128:(d0 + 3) * 128],
                             rhs=qT[:, 256:512], start=True, stop=True)
            fa = "act" if qs == 0 else None
            flex_exp(ptf[:, (d0 + 2) * 512 + 384:(d0 + 3) * 512], sps[:, 384:512], 128,
                     force=fa)
            flex_exp(ptf[:, (d0 + 3) * 512 + 256:(d0 + 4) * 512], sps[:, 768:1024], 384,
                     force=fa)
            emit_mask(pt, d0 + 2, 384)   # kb d0+3 boundary (j=3)
            emit_mask(pt, d0 + 3, 256)   # kb d0+2 boundary (j=2)
            pace(700.0)

        def emit_pv(h, qs, j, pt, po):
            qq = 4 * qs + j
            for kb in range(qq + 1):
                s_ = kb ^ 1
                nc.tensor.matmul(
                    po[:, j, :],
                    lhsT=pt[:, s_, j * 128:(j + 1) * 128],
                    rhs=vones_sb[:, s_, h, :],
                    start=(kb == 0), stop=(kb == qq),
                )

        def flush_pv(h, qs, pt, final=False):
            po = psa.tile([128, 4, HS + 1], F32, tag="o", bufs=2, name="po")
            rl = rlp.tile([128, 4], F32, tag="rl")
            for j in range(4):
                emit_pv(h, qs, j, pt, po)
            nc.vector.reciprocal(out=rl, in_=po[:, :, HS])
            load["dve"] += 190.0
            # normalize all 4 q-chunks in one broadcast multiply (DVE)
            nc.vector.tensor_mul(
                out=o_sb[:, h // 2, 4 * qs:4 * qs + 4, (h % 2) * HS:(h % 2 + 1) * HS],
                in0=po[:, :, 0:HS],
                in1=rl[:, :].broadcast_to([128, 4, HS]),
            )
            load["dve"] += 4 * HS * 1.042 + 125.0
            if h == HPC - 1:
                for c in range(EC // 128):
                    nc.sync.dma_start_transpose(
                        out=oT_sb[:, c, 4 * qs:4 * qs + 4, :],
                        in_=o_sb[:, c, 4 * qs:4 * qs + 4, :],
                    )
                if final:
                    for tt in range(4 * qs, 4 * qs + 4):
                        emit_outproj(tt)

        def emit_outproj(tt):
            ps = psa.tile([128, 1024], F32, tag="s", bufs=3, name="pso")
            for half in range(2):
                for c in range(EC // 128):
                    nc.tensor.matmul(
                        ps[:, half * 512:(half + 1) * 512],
                        lhsT=oT_sb[:, c, tt, :],
                        rhs=wo_sb[:, c, half * 512:(half + 1) * 512],
                        start=(c == 0), stop=(c == EC // 128 - 1),
                    )
            outsb = obp.tile([128, 1024], F16, tag="ob", name="outsb")
            flex(
                lambda: nc.scalar.activation(out=outsb, in_=ps, func=COPY),
                lambda: nc.vector.tensor_copy(out=outsb, in_=ps),
                1024,
            )
            # out-DMA on the SWDGE queue: interleaving plain DMAs between
            # batched XBAR transposes on the sync HWDGE queue faults the
            # DMA unit (NRT_EXEC_UNIT_UNRECOVERABLE)
            nc.gpsimd.dma_start(out=outp[tt * 128:(tt + 1) * 128, :], in_=outsb)

        # ---- schedule ----
        # DEFER env bisects overlap features: a=proj/V fillers mid-stream,
        # b=flush deferred one unit, c=outproj deferred one qs
        DEFER = os.environ.get("DEFER", "")

        pend_flush = []
        pend_out = []

        def drain(lst, n=None):
            k = len(lst) if n is None else min(n, len(lst))
            for _ in range(k):
                lst.pop(0)()

        # per-unit filler placement: quad qs must be emitted before the first
        # flush that reads it; qk pair ts before scores(qs=ts) at unit 4ts
        fillers = {u: [] for u in range(4 * NQS)}
        if "a" in DEFER:
            emit_qk_pair(0, 0)
            emit_qk_pair(1, 0)
            fillers[0] = [lambda: emit_v_quad(0), lambda: emit_qk_pair(0, 1)]
            fillers[1] = [lambda: emit_qk_pair(1, 1)]
            fillers[2] = [lambda: emit_v_quad(1)]
            fillers[4] = [lambda: emit_qk_pair(0, 2)]
            fillers[5] = [lambda: emit_qk_pair(1, 2)]
            fillers[6] = [lambda: emit_v_quad(2)]
            fillers[8] = [lambda: emit_qk_pair(0, 3)]
            fillers[9] = [lambda: emit_qk_pair(1, 3)]
            fillers[10] = [lambda: emit_v_quad(3)]
        else:
            for qk in range(2):
                for ts in range(NQS):
                    emit_qk_pair(qk, ts)
            for tq in range(NQS):
                emit_v_quad(tq)

        for qs in range(NQS):
            for h in range(HPC):
                unit = 4 * qs + h
                for f in fillers[unit]:
                    f()
                pt = ptp.tile([128, TT, QS], F16, tag="pT", name="pT")
                emit_scores(h, qs, pt, lambda x: None)
                if "b" in DEFER:
                    pend_flush.append(
                        lambda h=h, qs=qs, pt=pt: flush_pv(h, qs, pt, final=False))
                    if len(pend_flush) > 2:
                        drain(pend_flush, 1)
                else:
                    flush_pv(h, qs, pt, final=False)
                if "c" in DEFER:
                    drain(pend_out, 1)
            drain(pend_flush)
            if "c" in DEFER:
                for tt in range(4 * qs, 4 * qs + 4):
                    pend_out.append(lambda tt=tt: emit_outproj(tt))
            else:
                for tt in range(4 * qs, 4 * qs + 4):
                    emit_outproj(tt)
        drain(pend_out)


def build_nc(with_bias=False):
    nc = bacc.Bacc("TRN2", target_bir_lowering=False, debug=False)
    x8 = nc.dram_tensor("x8", (128, NQS * DC * QS), FP8, kind="ExternalInput")
    xl = nc.dram_tensor("xl", (128, NQS * DC * QS), FP8, kind="ExternalInput")
    xh = nc.dram_tensor("xh", (128, NQS * DC * QS), FP8, kind="ExternalInput")
    w8 = nc.dram_tensor("w8", (128, DC * 4 * 128), FP8, kind="ExternalInput")
    wql = nc.dram_tensor("wql", (128, DC * 4 * 128), FP8, kind="ExternalInput")
    wqh = nc.dram_tensor("wqh", (128, DC * 4 * 128), FP8, kind="ExternalInput")
    wv8 = nc.dram_tensor("wv8", (128, DC * EC), FP8, kind="ExternalInput")
    wvl = nc.dram_tensor("wvl", (128, DC * EC), FP8, kind="ExternalInput")
    wvh = nc.dram_tensor("wvh", (128, DC * EC), FP8, kind="ExternalInput")
    wo = nc.dram_tensor("wo", (128, (EC // 128) * D), F16, kind="ExternalInput")
    bqk = nc.dram_tensor("bqk", (128, 4), F32, kind="ExternalInput")
    mska = nc.dram_tensor("mska", (128, 128), F16, kind="ExternalInput")
    outp = nc.dram_tensor("outp", (T, D), F16, kind="ExternalOutput")
    with tile.TileContext(nc) as tc:
        _mha_tile_kernel(tc, outp[:], x8[:], xl[:], xh[:], w8[:], wql[:], wqh[:],
                         wv8[:], wvl[:], wvh[:], wo[:], bqk[:], mska[:], with_bias)
    nc.compile()
    return nc


def _e4(a):
    import ml_dtypes
    return np.clip(np.asarray(a, np.float32), -240.0, 240.0).astype(
        ml_dtypes.float8_e4m3)


def _fp8_split(a32):
    """a32 -> (hi, lo, hi16) with a32 ~= hi + lo/16 and hi16 = hi/16."""
    import ml_dtypes
    e4 = ml_dtypes.float8_e4m3
    s = np.clip(a32, -240.0, 240.0).astype(np.float32)
    hi = s.astype(e4)
    hif = hi.astype(np.float32)
    lo = np.clip(16.0 * (s - hif), -240.0, 240.0).astype(e4)
    hi16 = (hif / 16.0).astype(e4)
    return hi, lo, hi16


def make_in_maps(x, w_qkv, b_qkv, w_out):
    # causal boundary mask as fp16 0/1: msk[k, q] = 1 iff q >= k
    j = np.arange(128)
    mska = (j[:, None] <= j[None, :]).astype(np.float16)   # [k, q]

    # Q/K column layout: w8[p, c, et, i] = 32*w[c*128+p, col(et, i)] with
    # col(et, i) = base + g*EC + 128*(et%2) + i: et = (q/k)*2 + head-pair,
    # base = 0 (et<2: q) or D (et>=2: k); partitions carry (h%2)*64+hs
    i = np.arange(128)
    in_maps = []
    for c in range(NCORES):
        b, g = divmod(c, GROUPS)
        cols = np.empty((4, 128), np.int64)
        for et in range(4):
            base = 0 if et < 2 else D
            cols[et] = base + g * EC + 128 * (et % 2) + i
        wq = np.empty((128, DC, 4, 128), np.float32)
        for et in range(4):
            wq[:, :, et, :] = w_qkv[:, cols[et]].reshape(DC, 128, 128).transpose(1, 0, 2)
        wv = w_qkv[:, 2 * D + g * EC:2 * D + (g + 1) * EC].reshape(DC, 128, EC).transpose(1, 0, 2)
        wv8f, wvlf, wvhf = _fp8_split(wv)
        xT = np.ascontiguousarray(x[b].T)  # [D, T]
        x8f, xlf, xhf = _fp8_split(XS * xT)

        def _xlay(a):
            return np.ascontiguousarray(
                np.asarray(a).reshape(DC, 128, NQS, QS).transpose(1, 2, 0, 3)
            ).reshape(128, -1)

        bqk = np.empty((128, 4), np.float32)
        for et in range(4):
            bqk[:, et] = QS2 * b_qkv[cols[et]]
        wq8f, wqlf, wqhf = _fp8_split(WSQK * wq)
        in_maps.append({
            "x8": _xlay(x8f),
            "xl": _xlay(xlf),
            "xh": _xlay(xhf),
            "w8": np.ascontiguousarray(wq8f).reshape(128, -1),
            "wql": np.ascontiguousarray(wqlf).reshape(128, -1),
            "wqh": np.ascontiguousarray(wqhf).reshape(128, -1),
            "wv8": np.ascontiguousarray(wv8f).reshape(128, -1),
            "wvl": np.ascontiguousarray(wvlf).reshape(128, -1),
            "wvh": np.ascontiguousarray(wvhf).reshape(128, -1),
            "wo": np.ascontiguousarray(
                w_out[g * EC:(g + 1) * EC, :].reshape(EC // 128, 128, D)
                .transpose(1, 0, 2)).astype(np.float16).reshape(128, -1),
            "bqk": bqk,
            "mska": mska,
        })
    return in_maps


_NC_CACHE = {}


def get_nc(with_bias=False):
    key = f"nc{int(with_bias)}-{os.environ.get('DEFER', '')}-{os.environ.get('FEXP', '')}"
    if key not in _NC_CACHE:
        _NC_CACHE[key] = build_nc(with_bias)
    return _NC_CACHE[key]


def run_on_hw(in_maps, with_bias=False, **kwargs):
    nc = get_nc(with_bias)
    return bass_utils.run_bass_kernel_spmd(
        nc, in_maps, core_ids=list(range(NCORES)), **kwargs
    )


def kernel(x, w_qkv, b_qkv, w_out, b_out):
    x = np.asarray(x, dtype=np.float32)
    w_qkv = np.asarray(w_qkv, dtype=np.float32)
    b_qkv = np.asarray(b_qkv, dtype=np.float32)
    w_out = np.asarray(w_out, dtype=np.float32)
    b_out = np.asarray(b_out, dtype=np.float32)

    with_bias = bool(np.any(b_qkv[:2 * D] != 0.0))
    in_maps = make_in_maps(x, w_qkv, b_qkv, w_out)
    res = run_on_hw(in_maps, with_bias=with_bias)
    parts = [r["outp"].astype(np.float64) for r in res.results]
    out = np.stack([
        sum(parts[GROUPS * b:GROUPS * (b + 1)]) for b in range(B)
    ]).astype(np.float32)
    # exact V-bias fold: softmax weights sum to 1, so +b_v passes through
    # attention unchanged and lands as b_v @ w_out
    return out + (b_out + b_qkv[2 * D:] @ w_out)[None, None, :]


# revision 40
# speedup vs baseline: 1.1150x; 1.1150x over previous
# Multi-head causal attention (B=2, T=2048, D=1024, H=16, HS=64) on 8 TRN2 NeuronCores.
#
# Sharding: core c = (batch b = c//4, head-group g = c%4 -> heads 4g..4g+3).
# Host pre-transposes x, slices w_qkv columns / w_out rows per core; each core
# computes a partial (T, D) output projection and the host sums the 4 partials
# per batch (+ b_out).
#
# Device dataflow (per core):
#   QKV projections run in fp8(e4m3) DoubleRow mode with an exact 3-term
#   error-split (x = x_hi + x_lo, w = w_hi + w_lo, dropping only lo*lo):
#   hi*hi pairs two d-chunks per instruction; the two correction products of
#   each d-chunk ride the two DoubleRow k-tiles. Operands are host-prepared:
#   X8=Q(32x), XL=Q(16*(32x-X8)), XH=X8/16, W8=Q(32w), WH=W8/16,
#   WL=Q(16*(32w-W8)); all products sit at the same 1024*x*w scale, de-scaled
#   in the (DVE) bias-add.
#   Q^T,K^T [hs, t] come out of the projection in fp16; V lands natural [t,hs]
#   with a 65th column fixed at 1024.0 so the PV matmul yields both o_unnorm
#   and 1024*l while V itself carries psum + 1024*bias (scale cancels in o/l).
#   Scores are S^T [k, t] blocks; exp needs no max-subtraction (inputs ~N(0,1)).
#   exp is LOAD-BALANCED between ACT (native Exp) and DVE (fp16 Schraudolph:
#   bits16 = rne(s*1024*log2e/8 + 1024*(15-C)) written through a uint16
#   bitcast of the fp16 pt tile; tiny-p negative bits saturate to +0.0). The
#   t<512 q-rows (few softmax keys, no error averaging) stay on exact ACT.
#   P^T tiles are kb-indexed [128, 16, 512] so PV runs in the o = P^T.T @ V
#   orientation: out [q,65] costs 65 output columns per 128-key block instead
#   of 512. o is normalized per-q (reciprocal + broadcast along free dim),
#   transposed via the DMA XBAR (16x128 tiles, no PE/DVE cost) and fed to the
#   fp16 output projection; the psum drain copy is ACT/DVE load-balanced and
#   output DMAs ride the SWDGE queue.
import math
import os
import sys

import numpy as np

for _p in ("/opt/trn_rl_repo",):
    if _p not in sys.path and os.path.isdir(_p):
        sys.path.insert(0, _p)

import concourse.bass as bass
import concourse.mybir as mybir
import concourse.tile as tile
from concourse import bacc
from concourse import bass_utils

B, T, D = 2, 2048, 1024
H, HS = 16, 64
NCORES = 8
GROUPS = NCORES // B          # head-groups per batch = 4
HPC = H // GROUPS             # heads per core = 4
EC = HPC * HS                 # head-dim cols per section per core = 256
DC = D // 128                 # d-chunks = 8
TT = T // 128                 # t-tiles = 16
QS = 512                      # q-supertile
NQS = T // QS                 # 4
SCALE = 1.0 / math.sqrt(HS)

F32 = mybir.dt.float32
F16 = mybir.dt.float16
FP8 = mybir.dt.float8e4
U16 = mybir.dt.uint16
DR = mybir.MatmulPerfMode.DoubleRow
XS = 32.0                     # x fp8 pre-scale
WS = 32.0                     # w fp8 pre-scale
DESCALE = 1.0 / (XS * WS)

# fp16 Schraudolph exp for the DVE path: p = exp(s*SCALE), score psum in raw
# q*k units. uint16 convert saturates tiny-p negative bits to +0.0; C=0.043
# is the linear-mantissa minimax constant.
A16 = 1024.0 * math.log2(math.e) * SCALE
B16 = 1024.0 * (15.0 - 0.043)

PTLAG = 4                     # flush deadline in units (< pt pool bufs - 1)
MULT = mybir.AluOpType.mult
ADD = mybir.AluOpType.add


def _slot(kb, d0):
    # pT slot for key-block kb: diagonal blocks are pairwise swapped so each
    # exp's output region is contiguous in the flattened pT tile.
    if kb < d0:
        return kb
    return d0 + {0: 1, 1: 0, 2: 3, 3: 2}[kb - d0]


def _mha_tile_kernel(tc, outp, x8, xl, xh, w8, wh, wl, wo, bqk, bvb, mask):
    nc = tc.nc
    EXP = mybir.ActivationFunctionType.Exp

    # greedy pointwise load balance between ACT and DVE (ns of busy time)
    load = {"act": 0.0, "dve": 0.0}

    with (
        tc.tile_pool(name="singles", bufs=1) as singles,
        tc.tile_pool(name="pt", bufs=5) as ptp,
        tc.tile_pool(name="rl", bufs=4) as rlp,
        tc.tile_pool(name="ob", bufs=5) as obp,
        tc.tile_pool(name="psum", bufs=1, space="PSUM") as psa,
    ):
        # ---- loads: QK-critical pieces first, split across SP-HWDGE and
        # Pool-SWDGE so descriptor generation runs in parallel ----
        x8_sb = singles.tile([128, DC, T], FP8)
        xl_sb = singles.tile([128, DC, T], FP8)
        xh_sb = singles.tile([128, DC, T], FP8)
        w8_sb = singles.tile([128, DC, 3 * EC], FP8)
        wh_sb = singles.tile([128, DC, 3 * EC], FP8)
        wl_sb = singles.tile([128, DC, 3 * EC], FP8)
        wo_sb = singles.tile([128, EC // 128, D], F16)
        x8_r = x8.rearrange("(c p) t -> p c t", p=128)
        xl_r = xl.rearrange("(c p) t -> p c t", p=128)
        xh_r = xh.rearrange("(c p) t -> p c t", p=128)
        w8_r = w8.rearrange("(c p) e -> p c e", p=128)
        wh_r = wh.rearrange("(c p) e -> p c e", p=128)
        wl_r = wl.rearrange("(c p) e -> p c e", p=128)
        # QK-critical first: W slices for heads 0/1 (q cols 0:128, k cols
        # 256:384), x ts0 slabs in parallel on Pool-SWDGE; then h2/h3 W
        # slices, V columns, later x slabs, wo last.
        QK2 = 2 * EC
        bqk_sb = singles.tile([128, 4], F32)
        bvb_sb = singles.tile([1, EC], FP8)
        ones16_sb = singles.tile([1, 128], FP8)
        nc.vector.memset(ones16_sb, 16.0)
        mask_sb = singles.tile([128, 128], F16)

        nc.sync.dma_start(out=w8_sb[:, :, 0:QK2], in_=w8_r[:, :, 0:QK2])
        nc.gpsimd.dma_start(out=x8_sb[:, :, 0:QS], in_=x8_r[:, :, 0:QS])
        nc.sync.dma_start(out=bqk_sb, in_=bqk.rearrange("(c p) -> p c", p=128))
        nc.sync.dma_start(out=wh_sb[:, :, 0:QK2], in_=wh_r[:, :, 0:QK2])
        nc.gpsimd.dma_start(out=xl_sb[:, :, 0:QS], in_=xl_r[:, :, 0:QS])
        nc.sync.dma_start(out=wl_sb[:, :, 0:QK2], in_=wl_r[:, :, 0:QK2])
        nc.gpsimd.dma_start(out=xh_sb[:, :, 0:QS], in_=xh_r[:, :, 0:QS])
        nc.sync.dma_start(out=bvb_sb, in_=bvb.rearrange("(o e) -> o e", o=1))
        nc.sync.dma_start(out=mask_sb, in_=mask)
        nc.sync.dma_start(out=w8_sb[:, :, QK2:], in_=w8_r[:, :, QK2:])
        nc.sync.dma_start(out=wh_sb[:, :, QK2:], in_=wh_r[:, :, QK2:])
        nc.sync.dma_start(out=wl_sb[:, :, QK2:], in_=wl_r[:, :, QK2:])
        for ts in range(1, NQS):
            sl = slice(ts * QS, (ts + 1) * QS)
            nc.gpsimd.dma_start(out=x8_sb[:, :, sl], in_=x8_r[:, :, sl])
            nc.gpsimd.dma_start(out=xl_sb[:, :, sl], in_=xl_r[:, :, sl])
            nc.gpsimd.dma_start(out=xh_sb[:, :, sl], in_=xh_r[:, :, sl])
        nc.gpsimd.dma_start(out=wo_sb, in_=wo.rearrange("(c p) e -> p c e", p=128))

        qkT_sb = singles.tile([128, 4, T], F16)
        vones_sb = singles.tile([128, TT, HPC, HS + 1], F16)
        o_sb = singles.tile([128, TT, EC], F16)
        oT_sb = singles.tile([128, EC // 128, T], F16)
        nc.vector.memset(vones_sb[:, :, :, HS:HS + 1], XS * WS)

        def dr_group(ps, lhs_cols, rhs_cols, rhs_is_w, tail=0):
            # 12 DoubleRow matmuls: 4x hi*hi (paired d-chunks) + 8x corrections
            # (x_lo*w_hi and x_hi/16*16w_lo share one instruction per d-chunk).
            plan = (
                [(x8_sb, w8_sb, 2 * dp) for dp in range(DC // 2)]
                + [(xl_sb, wh_sb, None)] * (DC // 2)
                + [(xh_sb, wl_sb, None)] * (DC // 2)
            )
            for i, (xt, wt, _) in enumerate(plan):
                dc2 = (i % (DC // 2)) * 2
                xs_ap = xt[:, dc2:dc2 + 2, rhs_cols if not rhs_is_w else lhs_cols]
                ws_ap = wt[:, dc2:dc2 + 2, lhs_cols if not rhs_is_w else rhs_cols]
                if rhs_is_w:
                    lhsT, rhs = xs_ap, ws_ap
                else:
                    lhsT, rhs = ws_ap, xs_ap
                nc.tensor.matmul(
                    ps, lhsT=lhsT, rhs=rhs,
                    start=(i == 0), stop=(tail == 0 and i == len(plan) - 1),
                    perf_mode=DR,
                )

        def emit_qk(et, ts):
            ps = psa.tile([128, QS], F32, tag="s", bufs=3, name="psqk")
            dr_group(ps, slice(et * 128, (et + 1) * 128),
                     slice(ts * QS, (ts + 1) * QS), rhs_is_w=False)
            nc.vector.tensor_scalar(
                out=qkT_sb[:, et, ts * QS:(ts + 1) * QS],
                in0=ps, scalar1=DESCALE, scalar2=bqk_sb[:, et:et + 1],
                op0=MULT, op1=ADD,
            )
            load["dve"] += 512 * 1.042 + 125.0

        def emit_v(tt):
            ps = psa.tile([128, EC], F32, tag="s", bufs=3, name="psv")
            dr_group(ps, slice(tt * 128, (tt + 1) * 128),
                     slice(2 * EC, 3 * EC), rhs_is_w=True, tail=1)
            # bias row: 16.0 * (64*bv) = 1024*bv joins the psum group
            nc.tensor.matmul(ps, lhsT=ones16_sb, rhs=bvb_sb,
                             start=False, stop=True)
            nc.vector.tensor_copy(
                out=vones_sb[:, tt ^ 1, :, 0:HS],
                in_=ps.rearrange("p (h s) -> p h s", h=HPC),
            )
            load["dve"] += 256 * 1.042 + 125.0

        def flex_exp(pt_out, sps_in, cols, force=None):
            # exp on ACT (native) or DVE (fp16 Schraudolph), greedy-balanced
            ca = cols * 0.833 + 185.0
            cd = cols * 1.042 + 125.0
            eng = force or ("act" if load["act"] + ca <= load["dve"] + cd
                            else "dve")
            if eng == "act":
                load["act"] += ca
                nc.scalar.activation(out=pt_out, in_=sps_in, func=EXP,
                                     scale=SCALE)
            else:
                load["dve"] += cd
                nc.vector.tensor_scalar(
                    out=pt_out.bitcast(U16), in0=sps_in,
                    scalar1=A16, scalar2=B16, op0=MULT, op1=ADD)

        def emit_scores(h, qs, pt, pace):
            pb = 64 * (h % 2)
            qT = qkT_sb[pb:pb + 64, h // 2, qs * QS:(qs + 1) * QS]
            kT = qkT_sb[pb:pb + 64, 2 + h // 2, :]
            d0 = 4 * qs
            ptf = pt[:].rearrange("p a b -> p (a b)")
            # rows t<512 (qs==0) keep exact ACT exp: few softmax keys means
            # Schraudolph's ~4% sawtooth would not average out
            fa = "act" if qs == 0 else None

            for j2 in range(2 * qs):
                sps = psa.tile([128, 1024], F32, tag="s", bufs=3, name="sps")
                for half in range(2):
                    kb = 2 * j2 + (1 - half)  # slot s holds kb s^1
                    nc.tensor.matmul(
                        sps[:, half * 512:(half + 1) * 512],
                        lhsT=kT[:, kb * 128:(kb + 1) * 128], rhs=qT,
                        start=True, stop=True,
                    )
                flex_exp(pt[:, 2 * j2:2 * j2 + 2, :], sps, 1024)
                pace(1040.0)
            # diagonal pair A: slot d0 <- kb d0+1 (q cols 128:512),
            #                  slot d0+1 <- kb d0 (q cols 0:512)
            sps = psa.tile([128, 1024], F32, tag="s", bufs=3, name="sps")
            nc.tensor.matmul(sps[:, 128:512],
                             lhsT=kT[:, (d0 + 1) * 128:(d0 + 2) * 128],
                             rhs=qT[:, 128:512], start=True, stop=True)
            nc.tensor.matmul(sps[:, 512:1024],
                             lhsT=kT[:, d0 * 128:(d0 + 1) * 128],
                             rhs=qT, start=True, stop=True)
            flex_exp(ptf[:, d0 * 512 + 128:(d0 + 2) * 512], sps[:, 128:1024],
                     896, force=fa)
            pace(932.0)
            # diagonal pair B: slot d0+2 <- kb d0+3 (q 384:512),
            #                  slot d0+3 <- kb d0+2 (q 256:512)
            sps = psa.tile([128, 1024], F32, tag="s", bufs=3, name="sps")
            nc.tensor.matmul(sps[:, 384:512],
                             lhsT=kT[:, (d0 + 3) * 128:(d0 + 4) * 128],
                             rhs=qT[:, 384:512], start=True, stop=True)
            nc.tensor.matmul(sps[:, 512 + 256:1024],
                             lhsT=kT[:, (d0 + 2) * 128:(d0 + 3) * 128],
                             rhs=qT[:, 256:512], start=True, stop=True)
            flex_exp(ptf[:, (d0 + 2) * 512 + 384:(d0 + 3) * 512],
                     sps[:, 384:512], 128, force=fa)
            flex_exp(ptf[:, (d0 + 3) * 512 + 256:(d0 + 4) * 512],
                     sps[:, 768:1024], 384, force=fa)
            # mask the four diagonal boundary triangles
            for jp in range(4):
                s = _slot(d0 + jp, d0)
                nc.vector.tensor_mul(
                    out=pt[:, s, jp * 128:(jp + 1) * 128],
                    in0=pt[:, s, jp * 128:(jp + 1) * 128],
                    in1=mask_sb,
                )
            load["dve"] += 4 * (128 * 0.261 + 125.0)
            pace(718.0)

        def emit_pv(h, qs, j, pt, po):
            qq = 4 * qs + j
            for kb in range(qq + 1):
                s_ = kb ^ 1
                nc.tensor.matmul(
                    po[:, j, :],
                    lhsT=pt[:, s_, j * 128:(j + 1) * 128],
                    rhs=vones_sb[:, s_, h, :],
                    start=(kb == 0), stop=(kb == qq),
                )

        def flush_pv(h, qs, pt, final=False):
            # PV for all 4 q-chunks of this head + normalize; one po tile
            # (1 PSUM bank) holds the 4 j-regions.
            po = psa.tile([128, 4, HS + 1], F32, tag="o", bufs=2, name="po")
            rl = rlp.tile([128, 4], F32, tag="rl")
            for j in range(4):
                emit_pv(h, qs, j, pt, po)
            nc.vector.reciprocal(out=rl, in_=po[:, :, HS])
            load["dve"] += 190.0
            for j in range(4):
                nc.vector.tensor_scalar_mul(
                    out=o_sb[:, 4 * qs + j, h * HS:(h + 1) * HS],
                    in0=po[:, j, 0:HS],
                    scalar1=rl[:, j:j + 1],
                )
                load["dve"] += 64 * 1.042 + 125.0
                if h == HPC - 1:
                    tt = 4 * qs + j
                    for c in range(EC // 128):
                        nc.sync.dma_start_transpose(
                            out=oT_sb[:, c, tt * 128:(tt + 1) * 128],
                            in_=o_sb[:, tt, c * 128:(c + 1) * 128],
                        )
                    if final:
                        emit_outproj(tt)

        def emit_outproj(tt):
            ps = psa.tile([128, 1024], F32, tag="s", bufs=3, name="pso")
            for half in range(2):
                for c in range(EC // 128):
                    nc.tensor.matmul(
                        ps[:, half * 512:(half + 1) * 512],
                        lhsT=oT_sb[:, c, tt * 128:(tt + 1) * 128],
                        rhs=wo_sb[:, c, half * 512:(half + 1) * 512],
                        start=(c == 0), stop=(c == EC // 128 - 1),
                    )
            outsb = obp.tile([128, 1024], F16, tag="ob", name="outsb")
            ca = 1024 * 0.833 + 185.0
            cd = 1024 * 1.042 + 125.0
            if load["act"] + ca <= load["dve"] + cd:
                load["act"] += ca
                nc.scalar.copy(out=outsb, in_=ps)
            else:
                load["dve"] += cd
                nc.vector.tensor_copy(out=outsb, in_=ps)
            # SWDGE queue keeps plain DMAs off the XBAR-transpose HWDGE queue
            nc.gpsimd.dma_start(out=outp[tt * 128:(tt + 1) * 128, :], in_=outsb)

        # ---- schedule ----
        emit_qk(0, 0)
        emit_qk(2, 0)

        # ---- globally paced schedule: scores/exp units stream continuously;
        # PE-side fillers (proj, PV flushes, out-proj) are popped from a FIFO
        # in proportion to emitted exp time so the exp engines never starve.
        # Deadlines keep pool rotations sound. ----
        import collections as _c

        fq = _c.deque()        # items: [cost_ns, deadline_unit, closure]
        debt = [0.0]

        def fdrain(unit=None, all_=False):
            while fq and (all_ or (fq[0][1] is not None and fq[0][1] <= unit)):
                c, dl, f = fq.popleft()
                debt[0] = max(debt[0] - c, -3000.0)
                f()

        def pace(act_ns):
            debt[0] += act_ns * 0.6
            while fq and debt[0] > 0.0:
                c, dl, f = fq.popleft()
                debt[0] -= c
                f()

        def qflush(h, qs, pt, unit):
            def run():
                final = qs == NQS - 1 and h == HPC - 1
                flush_pv(h, qs, pt, final=final)
                if h == HPC - 1 and not final:
                    for tt in range(4 * qs, 4 * qs + 4):
                        fq.append([860.0, None, lambda tt=tt: emit_outproj(tt)])
            fq.append([300.0 + 260.0 * qs, unit + PTLAG, run])

        for et in (1, 3):
            fq.append([1290.0, 2, lambda et=et: emit_qk(et, 0)])
        for et in (0, 2, 1, 3):
            fq.append([1290.0, 4, lambda et=et: emit_qk(et, 1)])
        for tt in range(4):
            fq.append([710.0, 4, lambda tt=tt: emit_v(tt)])
        for tt in range(4, 8):
            fq.append([710.0, 4, lambda tt=tt: emit_v(tt)])
        for qs in range(NQS):
            if qs < NQS - 1 and qs >= 1:
                for et in (0, 2, 1, 3):
                    fq.append([1290.0, 4 * qs + 4,
                               lambda et=et, ts=qs + 1: emit_qk(et, ts)])
                for tt in range(4 * qs + 4, 4 * qs + 8):
                    fq.append([710.0, 4 * qs + 4, lambda tt=tt: emit_v(tt)])
            for h in range(HPC):
                unit = 4 * qs + h
                fdrain(unit=unit)
                pt = ptp.tile([128, TT, QS], F16, tag="pT", name="pT")
                emit_scores(h, qs, pt, pace)
                qflush(h, qs, pt, unit)
        fdrain(all_=True)


def build_nc():
    nc = bacc.Bacc("TRN2", target_bir_lowering=False, debug=False)
    x8 = nc.dram_tensor("x8", (D, T), FP8, kind="ExternalInput")
    xl = nc.dram_tensor("xl", (D, T), FP8, kind="ExternalInput")
    xh = nc.dram_tensor("xh", (D, T), FP8, kind="ExternalInput")
    w8 = nc.dram_tensor("w8", (D, 3 * EC), FP8, kind="ExternalInput")
    wh = nc.dram_tensor("wh", (D, 3 * EC), FP8, kind="ExternalInput")
    wl = nc.dram_tensor("wl", (D, 3 * EC), FP8, kind="ExternalInput")
    wo = nc.dram_tensor("wo", (EC, D), F16, kind="ExternalInput")
    bqk = nc.dram_tensor("bqk", (2 * EC,), F32, kind="ExternalInput")
    bvb = nc.dram_tensor("bvb", (EC,), FP8, kind="ExternalInput")
    mask = nc.dram_tensor("mask", (128, 128), F16, kind="ExternalInput")
    outp = nc.dram_tensor("outp", (T, D), F16, kind="ExternalOutput")
    with tile.TileContext(nc) as tc:
        _mha_tile_kernel(tc, outp[:], x8[:], xl[:], xh[:], w8[:], wh[:], wl[:],
                         wo[:], bqk[:], bvb[:], mask[:])
    nc.compile()
    return nc


def host_mask():
    # mask[p, c] = 1.0 where c >= p else 0 (fp16)
    p = np.arange(128)[:, None]
    c = np.arange(128)[None, :]
    return (c >= p).astype(np.float16)


def _e4(a):
    import ml_dtypes
    return np.clip(np.asarray(a, np.float32), -240.0, 240.0).astype(
        ml_dtypes.float8_e4m3)


def _fp8_split(a32, scale):
    """a32 (fp32) -> (hi8, lo8, hi16_8) with a*scale ~= hi + lo/16, hi16=hi/16."""
    import ml_dtypes
    e4 = ml_dtypes.float8_e4m3
    s = np.clip(a32 * scale, -240.0, 240.0).astype(np.float32)
    hi = s.astype(e4)
    hif = hi.astype(np.float32)
    lo = np.clip(16.0 * (s - hif), -240.0, 240.0).astype(e4)
    hi16 = (hif / 16.0).astype(e4)
    return hi, lo, hi16


def make_in_maps(x, w_qkv, b_qkv, w_out):
    mask = host_mask()
    in_maps = []
    for c in range(NCORES):
        b, g = divmod(c, GROUPS)
        cs = slice(EC * g, EC * (g + 1))
        wq_c = np.ascontiguousarray(
            np.concatenate(
                [w_qkv[:, cs], w_qkv[:, D:][:, cs], w_qkv[:, 2 * D:][:, cs]], axis=1
            )
        )
        xT = np.ascontiguousarray(x[b].T).astype(np.float32)
        x8, xl, xh = _fp8_split(xT, XS)
        w8, wl, wh = _fp8_split(wq_c, WS)
        in_maps.append({
            "x8": x8, "xl": xl, "xh": xh,
            "w8": w8, "wh": wh, "wl": wl,
            "wo": np.ascontiguousarray(w_out[cs, :]).astype(np.float16),
            "bqk": np.ascontiguousarray(
                np.concatenate([b_qkv[cs], b_qkv[D:][cs]])
            ).astype(np.float32),
            "bvb": _e4(64.0 * np.ascontiguousarray(b_qkv[2 * D:][cs])),
            "mask": mask,
        })
    return in_maps


_NC_CACHE = {}


def get_nc():
    if "nc" not in _NC_CACHE:
        _NC_CACHE["nc"] = build_nc()
    return _NC_CACHE["nc"]


def run_on_hw(in_maps, **kwargs):
    nc = get_nc()
    return bass_utils.run_bass_kernel_spmd(
        nc, in_maps, core_ids=list(range(NCORES)), **kwargs
    )


def kernel(x, w_qkv, b_qkv, w_out, b_out):
    x = np.asarray(x, dtype=np.float32)
    w_qkv = np.asarray(w_qkv, dtype=np.float32)
    b_qkv = np.asarray(b_qkv, dtype=np.float32)
    w_out = np.asarray(w_out, dtype=np.float32)
    b_out = np.asarray(b_out, dtype=np.float32)

    in_maps = make_in_maps(x, w_qkv, b_qkv, w_out)
    res = run_on_hw(in_maps)
    parts = [r["outp"].astype(np.float64) for r in res.results]
    out = np.stack([
        sum(parts[GROUPS * b:GROUPS * (b + 1)]) for b in range(B)
    ]).astype(np.float32)
    return out + b_out[None, None, :]
